# revision 34
# baseline (speedup 1.0000x reference)
"""Gaussian covariance kernel for Trainium2 (8 NeuronCores, SPMD).

Computes, per gaussian n:
    s = exp(scale[n])                  # [3]
    q = rot[n] / ||rot[n]||            # [4] quaternion (r,i,j,k)
    R = quat_to_rotmat(q)              # [3,3]
    Sigma[n] = (R*s) @ (R*s)^T         # [3,3]

Inputs : scale [4_000_000, 3] f32, rot [4_000_000, 4] f32
Output : [4_000_000, 3, 3] f32

Sharding: data-parallel over the gaussian dim across 8 cores
(500_000 each, padded to 500_096 = 128*3907 per core).

Math (scale-invariant, no normalize; everything at K/2 scale):
    n2   = |q|^2 (unnormalized)
    X_j  = (d+a, d+b, d+c);  Khat_jj = X_j - n2/2  (= K_jj/2)
    Khat_off = ij +- kr etc (UNdoubled products = K_off/2)
    wp_j = exp(s_j - ln n2)
    B    = Khat * diag(wp)  (one broadcast multiply)
    Sigma/4 = B @ B^T       (host multiplies the decoded output by 4)

Precision: bf16 inputs (host cast), fp32 n2/ln/exp chain on device,
bf16 everywhere else, bf16 output (host upcasts, *4).  End-to-end
L2 rel err 6.5e-3 (gate 2e-2).

HW shows ~1-2us fixed cost per DVE/Pool instruction, so the kernel is
built around instruction-count minimization:
 - host ships rot as SEVEN bf16 rows (qr, qi, qj, qk, qi, qj, -qj) so
   all 6 quaternion products take 2 instructions (cyclic windows):
     pa  = rot[1:4]*rot[2:5] = (ij, jk, ki)
     pb4 = rot[3:7]*bcast(qr) = (kr, ir, jr, -jr)
   and the off-diagonal K assembly takes 3 (row-major flat slots):
     kpA: (k10,k21) -> slots (3,7);  kmA: (k01,k12) -> slots (1,5)
     merged row2: (k02, k20) = bcast(ki) +- jr -> slots (2,6) via
     in1 = (jr, -jr)
 - K diagonal: slots (0,4,8) = X - bcast(n2/2)
 - B = K * wp broadcast over rows (one 9-unit op)
 - Sigma: U rows 0-2 = Square(B) (one ACT op), rows 3-5 = B_i*B_k
   pair products; two batched adds -> out chans 0..5; ACT copy 3..5
   -> 6..8.  Out channel order [S00,S11,S22,S01,S02,S12,S10,S20,S21].
 - big tiles share pool tags (K/U, rot/B, sq/dd, tm/out, X/pb4,
   scl/wp) so F=784 fits in SBUF with double buffering (5 tiles).
"""

import numpy as np
import ml_dtypes

N_TOTAL = 4_000_000
N_CORES = 8
N_PER_CORE = N_TOTAL // N_CORES          # 500_000
P = 128
L = 3907                                 # ceil(500_000/128) -> pad to 128*3907
N_PAD = P * L                            # 500_096
F_TILE = 512

BF16 = ml_dtypes.bfloat16

_STATE = {}


def _build_kernel(L=L, F_TILE=F_TILE, bufs=3,
                  dve=("X", "n2h", "kd", "tm", "rmul", "T0", "T1", "T2",
                       "ddall", "finall"),
                  share_tags=True):
    import concourse.bass as bass
    import concourse.bacc as bacc
    import concourse.tile as tile
    from concourse import mybir

    f32 = mybir.dt.float32
    bf16 = mybir.dt.bfloat16
    Alu = mybir.AluOpType
    Act = mybir.ActivationFunctionType

    nc = bacc.Bacc("TRN2", target_bir_lowering=False, debug=False,
                   num_devices=N_CORES)

    def eng(name):
        return nc.vector if name in dve else nc.gpsimd

    rot_d = nc.dram_tensor("rot", [P, 7 * L], bf16, kind="ExternalInput").ap() \
        .rearrange("p (c l) -> p c l", c=7)
    scl_d = nc.dram_tensor("scale", [P, 3 * L], bf16, kind="ExternalInput").ap() \
        .rearrange("p (c l) -> p c l", c=3)
    out_d = nc.dram_tensor("out", [P, 9 * L], bf16, kind="ExternalOutput").ap() \
        .rearrange("p (c l) -> p c l", c=9)

    bounds = []
    t0 = 0
    while t0 < L:
        f = min(F_TILE, L - t0)
        bounds.append((t0, f))
        t0 += f

    # Pin all activations (Ln, Exp, Square, Copy) to the one table that
    # holds them all; the default pass ping-pongs Ln/Exp table loads
    # every tile (~1.3us each).
    import concourse.bacc as bacc_mod
    from concourse.hw_specs import get_activation_tables

    def _patched_insert_act_table_loads():
        has_activation = any(
            isinstance(i, mybir.InstActivation)
            for b in nc.main_func.blocks
            for i in b.instructions
        )
        if not has_activation:
            return
        keep = "natural_log_exp_and_others"
        tables = [
            (nm, (s if nm == keep else set()))
            for nm, s in get_activation_tables(nc.m.arch).items()
        ]
        assert any(nm == keep and s for nm, s in tables)
        bacc_mod._bass_rust.insert_act_table_loads(nc, tables)

    nc.insert_act_table_loads = _patched_insert_act_table_loads

    def tag(primary, shared):
        return primary if share_tags else shared

    with tile.TileContext(nc) as tc, \
            nc.allow_low_precision("bf16 kernel, harness gate is 2e-2"):
        with tc.tile_pool(name="io", bufs=bufs) as io, \
             tc.tile_pool(name="tmp", bufs=bufs) as tp:
            for (t0, F) in bounds:
                # shared-tag pairs: first alloc of the pair happens early
                # in the tile, second later; with bufs=2 they alternate
                # buffers so lifetimes never collide.
                rot_t = io.tile([P, 7, F], bf16, tag="rotB", name="rot_t")
                scl_t = io.tile([P, 3, F], bf16, tag="sclwp", name="scl_t")
                nc.sync.dma_start(out=rot_t, in_=rot_d[:, :, t0:t0 + F])
                nc.sync.dma_start(out=scl_t, in_=scl_d[:, :, t0:t0 + F])

                # ---- fp32 side ----------------------------------------
                sq_t = tp.tile([P, 4, F], f32, tag="sqdd", name="sq_t")
                nc.scalar.activation(
                    out=sq_t.rearrange("p c f -> p (c f)"),
                    in_=rot_t[:, 0:4, :].rearrange("p c f -> p (c f)"),
                    func=Act.Square)

                X = tp.tile([P, 3, F], f32, tag="Xpb", name="X")
                eng("X").tensor_add(
                    out=X, in0=sq_t[:, 0:1, :].broadcast_to([P, 3, F]),
                    in1=sq_t[:, 1:4, :])
                uu = tp.tile([P, F], f32, tag="uu", name="uu")
                eng("u").tensor_add(out=uu, in0=sq_t[:, 2, :],
                                    in1=sq_t[:, 3, :])
                n2t = tp.tile([P, F], f32, tag="n2", name="n2t")
                eng("n2").tensor_add(out=n2t, in0=X[:, 0, :], in1=uu)
                n2h = tp.tile([P, F], f32, tag="n2h", name="n2h")
                eng("n2h").tensor_scalar_mul(out=n2h, in0=n2t, scalar1=0.5)

                lg = tp.tile([P, F], f32, tag="lg", name="lg")
                nc.scalar.activation(out=lg, in_=n2t, func=Act.Ln)
                tm = io.tile([P, 3, F], f32, tag="tmout", name="tm")
                eng("tm").tensor_sub(
                    out=tm, in0=scl_t,
                    in1=lg.rearrange("p (c f) -> p c f", c=1)
                         .broadcast_to([P, 3, F]))
                wp = io.tile([P, 3, F], bf16, tag="sclwp", name="wp")
                nc.scalar.activation(out=wp.rearrange("p c f -> p (c f)"),
                                     in_=tm.rearrange("p c f -> p (c f)"),
                                     func=Act.Exp)

                # ---- K/2 assembly (row-major [P, 3i, 3j, F]) ----------
                K = tp.tile([P, 3, 3, F], bf16, tag="KU", name="K")
                K9 = K.rearrange("p i j f -> p (i j) f")
                # diagonal slots (0,4,8): X_j - n2/2
                eng("kd").tensor_sub(
                    out=K9[:, 0:9:4, :], in0=X,
                    in1=n2h.rearrange("p (c f) -> p c f", c=1)
                           .broadcast_to([P, 3, F]))

                # products (cyclic 7-row layout)
                pa = tp.tile([P, 3, F], bf16, tag="pa", name="pa")
                pb4 = tp.tile([P, 4, F], bf16, tag="Xpb", name="pb4")
                eng("pa").tensor_mul(out=pa, in0=rot_t[:, 1:4, :],
                                     in1=rot_t[:, 2:5, :])
                eng("pb").tensor_mul(
                    out=pb4, in0=rot_t[:, 3:7, :],
                    in1=rot_t[:, 0:1, :].broadcast_to([P, 4, F]))

                # off-diagonals:
                # kpA: (k10,k21)=(ij+kr, jk+ir) -> slots (3,7)
                eng("kpA").tensor_add(out=K9[:, 3:8:4, :],
                                      in0=pa[:, 0:2, :], in1=pb4[:, 0:2, :])
                # kmA: (k01,k12)=(ij-kr, jk-ir) -> slots (1,5)
                eng("kmA").tensor_sub(out=K9[:, 1:6:4, :],
                                      in0=pa[:, 0:2, :], in1=pb4[:, 0:2, :])
                # merged row2: (k02,k20) = ki +- jr -> slots (2,6),
                # in1 = (jr, -jr)
                eng("kpB").tensor_add(
                    out=K9[:, 2:7:4, :],
                    in0=pa[:, 2:3, :].broadcast_to([P, 2, F]),
                    in1=pb4[:, 2:4, :])

                # ---- B = Khat * wp ------------------------------------
                B = io.tile([P, 3, 3, F], bf16, tag="rotB", name="B")
                eng("rmul").tensor_mul(
                    out=B, in0=K,
                    in1=wp.rearrange("p (o j) f -> p o j f", o=1)
                          .broadcast_to([P, 3, 3, F]))

                # ---- Sigma/4 = B B^T ----------------------------------
                U = tp.tile([P, 6, 3, F], bf16, tag="KU", name="U")
                nc.scalar.activation(
                    out=U[:, 0:3, :, :].rearrange("p i j f -> p (i j f)"),
                    in_=B.rearrange("p i j f -> p (i j f)"), func=Act.Square)
                for p_, (i_, k_) in enumerate([(0, 1), (0, 2), (1, 2)]):
                    eng(f"T{p_}").tensor_mul(out=U[:, 3 + p_, :, :],
                                             in0=B[:, i_, :, :],
                                             in1=B[:, k_, :, :])
                dd = tp.tile([P, 6, F], bf16, tag="sqdd", name="dd")
                eng("ddall").tensor_add(out=dd, in0=U[:, :, 0, :],
                                        in1=U[:, :, 1, :])
                out_t = io.tile([P, 9, F], bf16, tag="tmout", name="out_t")
                eng("finall").tensor_add(out=out_t[:, 0:6, :], in0=dd,
                                         in1=U[:, :, 2, :])
                # symmetric lower entries
                nc.scalar.copy(out=out_t[:, 6:9, :], in_=out_t[:, 3:6, :])

                nc.sync.dma_start(out=out_d[:, :, t0:t0 + F], in_=out_t)

    nc.compile()
    return nc


def _get_nc():
    if "nc" not in _STATE:
        _STATE["nc"] = _build_kernel()
    return _STATE["nc"]


def kernel(scale: np.ndarray, rot: np.ndarray) -> np.ndarray:
    from concourse.bass_utils import run_bass_kernel_spmd

    scale = np.asarray(scale, dtype=np.float32)
    rot = np.asarray(rot, dtype=np.float32)

    nc = _get_nc()

    in_maps = []
    for c in range(N_CORES):
        s = np.zeros((N_PAD, 3), np.float32)
        s[:N_PER_CORE] = scale[c * N_PER_CORE:(c + 1) * N_PER_CORE]
        r = np.zeros((N_PAD, 4), np.float32)
        r[:N_PER_CORE] = rot[c * N_PER_CORE:(c + 1) * N_PER_CORE]
        r[N_PER_CORE:, 0] = 1.0
        # [P, C, L] component-major; rot extended to 7 rows:
        # (qr, qi, qj, qk, qi, qj, -qj)
        r4 = np.ascontiguousarray(
            r.reshape(P, L, 4).transpose(0, 2, 1))          # [P,4,L]
        r7 = np.empty((P, 7, L), np.float32)
        r7[:, 0:4] = r4
        r7[:, 4] = r4[:, 1]
        r7[:, 5] = r4[:, 2]
        r7[:, 6] = -r4[:, 2]
        sb = np.ascontiguousarray(
            s.reshape(P, L, 3).transpose(0, 2, 1)).astype(BF16)
        in_maps.append({
            "scale": sb.reshape(P, 3 * L),
            "rot": r7.astype(BF16).reshape(P, 7 * L),
        })

    res = run_bass_kernel_spmd(nc, in_maps, core_ids=list(range(N_CORES)))

    # device channel order: [S00,S11,S22,S01,S02,S12,S10,S20,S21] (all /4)
    perm = [0, 3, 4, 6, 1, 5, 7, 8, 2]
    out = np.empty((N_TOTAL, 9), np.float32)
    for c in range(N_CORES):
        o = res.results[c]["out"].reshape(P, 9, L)[:, perm, :].transpose(0, 2, 1)
        out[c * N_PER_CORE:(c + 1) * N_PER_CORE] = (
            o.reshape(N_PAD, 9)[:N_PER_CORE].astype(np.float32))
    out *= 4.0
    return out.reshape(N_TOTAL, 3, 3)


# revision 35
# speedup vs baseline: 1.0155x; 1.0155x over previous
"""Gaussian covariance kernel for Trainium2 (8 NeuronCores, SPMD).

Computes, per gaussian n:
    s = exp(scale[n])                  # [3]
    q = rot[n] / ||rot[n]||            # [4] quaternion (r,i,j,k)
    R = quat_to_rotmat(q)              # [3,3]
    Sigma[n] = (R*s) @ (R*s)^T         # [3,3]

Inputs : scale [4_000_000, 3] f32, rot [4_000_000, 4] f32
Output : [4_000_000, 3, 3] f32

Sharding: data-parallel over the gaussian dim across 8 cores
(500_000 each, padded to 500_096 = 128*3907 per core).

Math (scale-invariant, no normalize; everything at K/2 scale):
    n2   = |q|^2 (unnormalized)
    X_j  = (d+a, d+b, d+c);  Khat_jj = X_j - n2/2  (= K_jj/2)
    Khat_off = ij +- kr etc (UNdoubled products = K_off/2)
    wp_j = exp(s_j - ln n2)
    B    = Khat * diag(wp)  (one broadcast multiply)
    Sigma/4 = B @ B^T       (host multiplies the decoded output by 4)

Precision: bf16 inputs (host cast), fp32 n2/ln/exp chain on device,
bf16 everywhere else, bf16 output (host upcasts, *4).  End-to-end
L2 rel err 6.5e-3 (gate 2e-2).

HW shows ~1-2us fixed cost per DVE/Pool instruction, so the kernel is
built around instruction-count minimization:
 - host ships rot as SEVEN bf16 rows (qr, qi, qj, qk, qi, qj, -qj) so
   all 6 quaternion products take 2 instructions (cyclic windows):
     pa  = rot[1:4]*rot[2:5] = (ij, jk, ki)
     pb4 = rot[3:7]*bcast(qr) = (kr, ir, jr, -jr)
   and the off-diagonal K assembly takes 3 (row-major flat slots):
     kpA: (k10,k21) -> slots (3,7);  kmA: (k01,k12) -> slots (1,5)
     merged row2: (k02, k20) = bcast(ki) +- jr -> slots (2,6) via
     in1 = (jr, -jr)
 - K diagonal: slots (0,4,8) = X - bcast(n2/2)
 - B = K * wp broadcast over rows (one 9-unit op)
 - Sigma: U rows 0-2 = Square(B) (one ACT op), rows 3-5 = B_i*B_k
   pair products; two batched adds -> out chans 0..5; ACT copy 3..5
   -> 6..8.  Out channel order [S00,S11,S22,S01,S02,S12,S10,S20,S21].
 - big tiles share pool tags (K/U, rot/B, sq/dd, tm/out, X/pb4,
   scl/wp) so F=784 fits in SBUF with double buffering (5 tiles).
"""

import numpy as np
import ml_dtypes

N_TOTAL = 4_000_000
N_CORES = 8
N_PER_CORE = N_TOTAL // N_CORES          # 500_000
P = 128
L = 3907                                 # ceil(500_000/128) -> pad to 128*3907
N_PAD = P * L                            # 500_096
F_TILE = 512

BF16 = ml_dtypes.bfloat16

_STATE = {}


def _build_kernel(L=L, F_TILE=F_TILE, bufs=2,
                  dve=("X", "u", "n2", "n2h", "kd", "tm", "rmul",
                       "T0", "T1", "T2", "ddall", "finall"),
                  share_tags=True):
    import concourse.bass as bass
    import concourse.bacc as bacc
    import concourse.tile as tile
    from concourse import mybir

    f32 = mybir.dt.float32
    bf16 = mybir.dt.bfloat16
    Alu = mybir.AluOpType
    Act = mybir.ActivationFunctionType

    nc = bacc.Bacc("TRN2", target_bir_lowering=False, debug=False,
                   num_devices=N_CORES)

    def eng(name):
        return nc.vector if name in dve else nc.gpsimd

    rot_d = nc.dram_tensor("rot", [P, 7 * L], bf16, kind="ExternalInput").ap() \
        .rearrange("p (c l) -> p c l", c=7)
    scl_d = nc.dram_tensor("scale", [P, 3 * L], bf16, kind="ExternalInput").ap() \
        .rearrange("p (c l) -> p c l", c=3)
    out_d = nc.dram_tensor("out", [P, 9 * L], bf16, kind="ExternalOutput").ap() \
        .rearrange("p (c l) -> p c l", c=9)

    bounds = []
    t0 = 0
    while t0 < L:
        f = min(F_TILE, L - t0)
        bounds.append((t0, f))
        t0 += f

    # Pin all activations (Ln, Exp, Square, Copy) to the one table that
    # holds them all; the default pass ping-pongs Ln/Exp table loads
    # every tile (~1.3us each).
    import concourse.bacc as bacc_mod
    from concourse.hw_specs import get_activation_tables

    def _patched_insert_act_table_loads():
        has_activation = any(
            isinstance(i, mybir.InstActivation)
            for b in nc.main_func.blocks
            for i in b.instructions
        )
        if not has_activation:
            return
        keep = "natural_log_exp_and_others"
        tables = [
            (nm, (s if nm == keep else set()))
            for nm, s in get_activation_tables(nc.m.arch).items()
        ]
        assert any(nm == keep and s for nm, s in tables)
        bacc_mod._bass_rust.insert_act_table_loads(nc, tables)

    nc.insert_act_table_loads = _patched_insert_act_table_loads

    def tag(primary, shared):
        return primary if share_tags else shared

    with tile.TileContext(nc) as tc, \
            nc.allow_low_precision("bf16 kernel, harness gate is 2e-2"):
        with tc.tile_pool(name="io", bufs=bufs) as io, \
             tc.tile_pool(name="tmp", bufs=bufs) as tp:
            for (t0, F) in bounds:
                # shared-tag pairs: first alloc of the pair happens early
                # in the tile, second later; with bufs=2 they alternate
                # buffers so lifetimes never collide.
                rot_t = io.tile([P, 7, F], bf16, tag="rotB", name="rot_t")
                scl_t = io.tile([P, 3, F], bf16, tag="sclwp", name="scl_t")
                nc.sync.dma_start(out=rot_t, in_=rot_d[:, :, t0:t0 + F])
                nc.sync.dma_start(out=scl_t, in_=scl_d[:, :, t0:t0 + F])

                # ---- fp32 side ----------------------------------------
                sq_t = tp.tile([P, 4, F], f32, tag="sqdd", name="sq_t")
                nc.scalar.activation(
                    out=sq_t.rearrange("p c f -> p (c f)"),
                    in_=rot_t[:, 0:4, :].rearrange("p c f -> p (c f)"),
                    func=Act.Square)

                X = tp.tile([P, 3, F], f32, tag="Xpb", name="X")
                eng("X").tensor_add(
                    out=X, in0=sq_t[:, 0:1, :].broadcast_to([P, 3, F]),
                    in1=sq_t[:, 1:4, :])
                uu = tp.tile([P, F], f32, tag="uu", name="uu")
                eng("u").tensor_add(out=uu, in0=sq_t[:, 2, :],
                                    in1=sq_t[:, 3, :])
                n2t = tp.tile([P, F], f32, tag="n2", name="n2t")
                eng("n2").tensor_add(out=n2t, in0=X[:, 0, :], in1=uu)
                n2h = tp.tile([P, F], f32, tag="n2h", name="n2h")
                eng("n2h").tensor_scalar_mul(out=n2h, in0=n2t, scalar1=0.5)

                lg = tp.tile([P, F], f32, tag="lg", name="lg")
                nc.scalar.activation(out=lg, in_=n2t, func=Act.Ln)
                tm = io.tile([P, 3, F], f32, tag="tmout", name="tm")
                eng("tm").tensor_sub(
                    out=tm, in0=scl_t,
                    in1=lg.rearrange("p (c f) -> p c f", c=1)
                         .broadcast_to([P, 3, F]))
                wp = io.tile([P, 3, F], bf16, tag="sclwp", name="wp")
                nc.scalar.activation(out=wp.rearrange("p c f -> p (c f)"),
                                     in_=tm.rearrange("p c f -> p (c f)"),
                                     func=Act.Exp)

                # ---- K/2 assembly (row-major [P, 3i, 3j, F]) ----------
                K = tp.tile([P, 3, 3, F], bf16, tag="KU", name="K")
                K9 = K.rearrange("p i j f -> p (i j) f")
                # diagonal slots (0,4,8): X_j - n2/2
                eng("kd").tensor_sub(
                    out=K9[:, 0:9:4, :], in0=X,
                    in1=n2h.rearrange("p (c f) -> p c f", c=1)
                           .broadcast_to([P, 3, F]))

                # products (cyclic 7-row layout)
                pa = tp.tile([P, 3, F], bf16, tag="pa", name="pa")
                pb4 = tp.tile([P, 4, F], bf16, tag="Xpb", name="pb4")
                eng("pa").tensor_mul(out=pa, in0=rot_t[:, 1:4, :],
                                     in1=rot_t[:, 2:5, :])
                eng("pb").tensor_mul(
                    out=pb4, in0=rot_t[:, 3:7, :],
                    in1=rot_t[:, 0:1, :].broadcast_to([P, 4, F]))

                # off-diagonals:
                # kpA: (k10,k21)=(ij+kr, jk+ir) -> slots (3,7)
                eng("kpA").tensor_add(out=K9[:, 3:8:4, :],
                                      in0=pa[:, 0:2, :], in1=pb4[:, 0:2, :])
                # kmA: (k01,k12)=(ij-kr, jk-ir) -> slots (1,5)
                eng("kmA").tensor_sub(out=K9[:, 1:6:4, :],
                                      in0=pa[:, 0:2, :], in1=pb4[:, 0:2, :])
                # merged row2: (k02,k20) = ki +- jr -> slots (2,6),
                # in1 = (jr, -jr)
                eng("kpB").tensor_add(
                    out=K9[:, 2:7:4, :],
                    in0=pa[:, 2:3, :].broadcast_to([P, 2, F]),
                    in1=pb4[:, 2:4, :])

                # ---- B = Khat * wp ------------------------------------
                B = io.tile([P, 3, 3, F], bf16, tag="rotB", name="B")
                eng("rmul").tensor_mul(
                    out=B, in0=K,
                    in1=wp.rearrange("p (o j) f -> p o j f", o=1)
                          .broadcast_to([P, 3, 3, F]))

                # ---- Sigma/4 = B B^T ----------------------------------
                U = tp.tile([P, 6, 3, F], bf16, tag="KU", name="U")
                nc.scalar.activation(
                    out=U[:, 0:3, :, :].rearrange("p i j f -> p (i j f)"),
                    in_=B.rearrange("p i j f -> p (i j f)"), func=Act.Square)
                for p_, (i_, k_) in enumerate([(0, 1), (0, 2), (1, 2)]):
                    eng(f"T{p_}").tensor_mul(out=U[:, 3 + p_, :, :],
                                             in0=B[:, i_, :, :],
                                             in1=B[:, k_, :, :])
                dd = tp.tile([P, 6, F], bf16, tag="sqdd", name="dd")
                eng("ddall").tensor_add(out=dd, in0=U[:, :, 0, :],
                                        in1=U[:, :, 1, :])
                out_t = io.tile([P, 9, F], bf16, tag="tmout", name="out_t")
                eng("finall").tensor_add(out=out_t[:, 0:6, :], in0=dd,
                                         in1=U[:, :, 2, :])
                # symmetric lower entries
                nc.scalar.copy(out=out_t[:, 6:9, :], in_=out_t[:, 3:6, :])

                nc.sync.dma_start(out=out_d[:, :, t0:t0 + F], in_=out_t)

    nc.compile()
    return nc


def _get_nc():
    if "nc" not in _STATE:
        _STATE["nc"] = _build_kernel()
    return _STATE["nc"]


def kernel(scale: np.ndarray, rot: np.ndarray) -> np.ndarray:
    from concourse.bass_utils import run_bass_kernel_spmd

    scale = np.asarray(scale, dtype=np.float32)
    rot = np.asarray(rot, dtype=np.float32)

    nc = _get_nc()

    in_maps = []
    for c in range(N_CORES):
        s = np.zeros((N_PAD, 3), np.float32)
        s[:N_PER_CORE] = scale[c * N_PER_CORE:(c + 1) * N_PER_CORE]
        r = np.zeros((N_PAD, 4), np.float32)
        r[:N_PER_CORE] = rot[c * N_PER_CORE:(c + 1) * N_PER_CORE]
        r[N_PER_CORE:, 0] = 1.0
        # [P, C, L] component-major; rot extended to 7 rows:
        # (qr, qi, qj, qk, qi, qj, -qj)
        r4 = np.ascontiguousarray(
            r.reshape(P, L, 4).transpose(0, 2, 1))          # [P,4,L]
        r7 = np.empty((P, 7, L), np.float32)
        r7[:, 0:4] = r4
        r7[:, 4] = r4[:, 1]
        r7[:, 5] = r4[:, 2]
        r7[:, 6] = -r4[:, 2]
        sb = np.ascontiguousarray(
            s.reshape(P, L, 3).transpose(0, 2, 1)).astype(BF16)
        in_maps.append({
            "scale": sb.reshape(P, 3 * L),
            "rot": r7.astype(BF16).reshape(P, 7 * L),
        })

    res = run_bass_kernel_spmd(nc, in_maps, core_ids=list(range(N_CORES)))

    # device channel order: [S00,S11,S22,S01,S02,S12,S10,S20,S21] (all /4)
    perm = [0, 3, 4, 6, 1, 5, 7, 8, 2]
    out = np.empty((N_TOTAL, 9), np.float32)
    for c in range(N_CORES):
        o = res.results[c]["out"].reshape(P, 9, L)[:, perm, :].transpose(0, 2, 1)
        out[c * N_PER_CORE:(c + 1) * N_PER_CORE] = (
            o.reshape(N_PAD, 9)[:N_PER_CORE].astype(np.float32))
    out *= 4.0
    return out.reshape(N_TOTAL, 3, 3)
